# revision 15
# baseline (speedup 1.0000x reference)
"""ComplexAttention (B=2, T=2048, D=1024, H=16, Dh=64) on 8 TRN2 NeuronCores.

Sharding: core c -> batch b = c // 4, heads [4*(c%4), 4*(c%4)+4).
Each core computes its 4 heads' QKV projections (column-sharded), causal
complex attention, and a partial output projection (row-sharded). The host
sums the 4 partials per batch and adds the output bias.

v2 (bf16): all matmuls run in bf16 (1 cyc/row on PE vs 4 for fp32; psum
accumulation stays fp32), x is transposed to [D, T] and downcast on the
host, and all bf16 operands ship in ONE pre-laid-out SBUF-image blob per
core (single big DMA, 3 input tensors total). Q/K projections pack two
heads per matmul (M=128 instead of 64), and Q/K/V stay SBUF-resident
(the fp32 version round-tripped Q/K through DRAM scratch).

Math notes:
  score = (qr kr^T + qi ki^T) / 8  ==  Qc Kc^T / 8  with Qc = [qr; qi] (128-d)
  -> contraction dim is exactly 128 = full PE partition dim.
  Attention is computed in the transposed domain: S^T[ktok, qtok] tiles,
  exp on ACT (no max subtraction needed: |S| <~ 3), causal mask via
  affine_select, unnormalized O^T accumulation on PE, row sums l via
  ones-matmul, normalization by 1/l broadcast with a K=1 matmul.
"""

import math
from contextlib import ExitStack

import numpy as np

import concourse.bass as bass
import concourse.tile as tile
from concourse import bacc, mybir
from concourse.bass_utils import run_bass_kernel_spmd

F32 = mybir.dt.float32
BF16 = mybir.dt.bfloat16

# Full-problem config (hardcoded per harness contract).
T = 2048
D = 1024
HPC = 4            # heads per core
DH = 64
QCH = 512          # query chunk (psum bank = 512 fp32)
N_CORES = 8
B = 2
H_TOTAL = 16

P = 128
DT = D // P        # 8 din tiles
KT = T // P        # 16 key tiles
QC = T // QCH      # 4 query chunks
QKB = QCH // P     # 4 key tiles per query chunk step
CW = HPC * DH      # 256 per-core qkv width
NPAIR = HPC // 2   # 2 head pairs

# blob layout: per-partition offsets (bf16 elems); see make_core_inputs
OFF_XR = 0
OFF_XI = OFF_XR + DT * T        # 16384
OFF_W = OFF_XI + DT * T         # 32768; 6 qkv weights, DT*CW each
OFF_WO = OFF_W + 6 * DT * CW    # 45056; 2 wo, NPAIR*D each
BLOB_N = OFF_WO + 2 * NPAIR * D  # 49152

# Flipped by test.py for profiling; harness path keeps these defaults.
TRACE = False
LAST = {}

CFG = dict(T=T, D=D, HPC=HPC, DH=DH, QCH=QCH)  # kept for test.py compat


def build_program(cfg=None, num_devices=N_CORES, enable_asserts=False,
                  phases=(0, 1, 2, 3)):
    scale = 1.0 / math.sqrt(DH)

    nc = bacc.Bacc(
        "TRN2",
        target_bir_lowering=False,
        debug=False,
        enable_asserts=enable_asserts,
        num_devices=num_devices,
    )

    # ---- DRAM I/O ----
    blob = nc.dram_tensor("blob", [P, BLOB_N], BF16, kind="ExternalInput").ap()
    bqk = nc.dram_tensor("bqk", [P, 2 * HPC], F32, kind="ExternalInput").ap()
    bv = nc.dram_tensor("bv", [1, 2 * CW], F32, kind="ExternalInput").ap()
    # partial sums ship as bf16 (halves the output DMA; the host upcasts to
    # fp32 before summing the 4 per-batch partials, so only one rounding)
    out_r = nc.dram_tensor("out_r", [T, D], BF16, kind="ExternalOutput").ap()
    out_i = nc.dram_tensor("out_i", [T, D], BF16, kind="ExternalOutput").ap()

    out_r_t = out_r.rearrange("(n p) d -> p n d", p=P)
    out_i_t = out_i.rearrange("(n p) d -> p n d", p=P)

    with tile.TileContext(nc) as tc, ExitStack() as octx:
        const = octx.enter_context(tc.tile_pool(name="const", bufs=1))
        opool = octx.enter_context(tc.tile_pool(name="opool", bufs=1))

        # blob slices (bf16 elem offsets, all 2D views). The DMA is split
        # so consumers only wait for their own region: weights first (small),
        # then x in 512-token chunks — Q/K projection of chunk c starts as
        # soon as chunk c lands instead of after the full 12MB transfer.
        bsb = opool.tile([P, BLOB_N], BF16, name="bsb")
        nc.sync.dma_start(bsb[:, OFF_W:BLOB_N], blob[:, OFF_W:BLOB_N])
        bsb_x = bsb[:, OFF_XR:OFF_W].rearrange("p (v d t) -> p v d t", v=2, d=DT)
        blob_x = blob[:, OFF_XR:OFF_W].rearrange("p (v d t) -> p v d t", v=2, d=DT)
        for c in range(2):  # 1024-col halves: 2KB descriptor lines (full eff)
            cs = slice(c * (T // 2), (c + 1) * (T // 2))
            nc.sync.dma_start(bsb_x[:, :, :, cs], blob_x[:, :, :, cs])

        def xr(d, lo, n):
            return bsb[:, OFF_XR + d * T + lo:OFF_XR + d * T + lo + n]

        def xi(d, lo, n):
            return bsb[:, OFF_XI + d * T + lo:OFF_XI + d * T + lo + n]

        def w(idx, d, lo, n):  # idx: 0 wq_r, 1 wq_i, 2 wk_r, 3 wk_i, 4 wv_r, 5 wv_i
            o = OFF_W + idx * DT * CW + d * CW + lo
            return bsb[:, o:o + n]

        def wo(idx, kk, lo, n):  # idx: 0 wo_r, 1 wo_i (host pair-permuted)
            o = OFF_WO + idx * NPAIR * D + kk * D + lo
            return bsb[:, o:o + n]

        # constants / biases
        ones_st = const.tile([P, P], F32)
        nc.vector.memset(ones_st, 1.0)
        ones_row = const.tile([1, P], F32)   # K=1 bcast lhsT (f32)
        nc.scalar.activation(ones_row, ones_st[0:1, :],
                             mybir.ActivationFunctionType.Copy)
        ones_col = const.tile([P, 1], BF16)  # lhsT for l = ones^T @ expS
        nc.scalar.activation(ones_col, ones_st[:, 0:1],
                             mybir.ActivationFunctionType.Copy)
        bqk_sb = const.tile([P, 2 * HPC], F32)
        nc.sync.dma_start(bqk_sb, bqk)
        bv_sb = const.tile([1, 2 * CW], F32)
        nc.sync.dma_start(bv_sb, bv)

        # Q/K/V SBUF-resident (flat free dims)
        q_sb = opool.tile([P, HPC * T], BF16, name="q_sb")
        k_sb = opool.tile([P, HPC * T], BF16, name="k_sb")
        vr_sb = opool.tile([P, KT * CW], BF16, name="vr_sb")
        vi_sb = opool.tile([P, KT * CW], BF16, name="vi_sb")
        # O^T head-pair blocks (bf16), into phase 3.
        # ORT[pair] rows: [vr_h_even(64) ; vr_h_odd(64)]
        # OIT[pair] rows: [vi_h_odd(64) ; vi_h_even(64)]  (host permutes wo_i)
        ort = [opool.tile([P, T], BF16, name=f"ort{p}") for p in range(NPAIR)]
        oit = [opool.tile([P, T], BF16, name=f"oit{p}") for p in range(NPAIR)]

        # ================= Phase 0: broadcast V bias =================
        with ExitStack() as ctx:
            ps_bc = ctx.enter_context(tc.tile_pool(name="ps_bc", bufs=2,
                                                   space="PSUM"))
            bvr_bc = const.tile([P, CW], F32)
            bvi_bc = const.tile([P, CW], F32)
            for (dst, lo) in ((bvr_bc, 0), (bvi_bc, CW)):
                pbc = ps_bc.tile([P, CW], F32, name="pbc", tag="pbc")
                nc.tensor.matmul(pbc, ones_row, bv_sb[:, lo:lo + CW],
                                 start=True, stop=True)
                nc.any.tensor_copy(out=dst, in_=pbc)

        # ================= Phase 1: projections =================
        with ExitStack() as ctx:
            ps_qk = ctx.enter_context(tc.tile_pool(name="ps_qk", bufs=2,
                                                   space="PSUM"))
            ps_v = ctx.enter_context(tc.tile_pool(name="ps_v", bufs=2,
                                                  space="PSUM"))

            # Q/K: two heads per matmul (M=128), rows [x_h0(64); x_h1(64)].
            # psR accumulates the real-weight path, psI the imag path;
            # head h=2*pr+lo takes psR[64lo:64lo+64] (qr) and psI[...] (qi).
            for c in range(QC):
                cl = c * QCH
                for pr in range(NPAIR):
                    for (wri, bofs, dst) in ((0, 0, q_sb), (2, HPC, k_sb)):
                        psR = ps_qk.tile([P, QCH], F32, name="psR", tag="psR")
                        psI = ps_qk.tile([P, QCH], F32, name="psI", tag="psI")
                        for d in range(DT):
                            nc.tensor.matmul(
                                psR, w(wri, d, pr * P, P), xr(d, cl, QCH),
                                start=(d == 0), stop=(d == DT - 1))
                            nc.tensor.matmul(
                                psI, w(wri + 1, d, pr * P, P), xi(d, cl, QCH),
                                start=(d == 0), stop=(d == DT - 1))
                        for lo in (0, 1):
                            h = 2 * pr + lo
                            hb = bofs + h
                            nc.any.tensor_scalar_add(
                                out=dst[0:64, h * T + cl:h * T + cl + QCH],
                                in0=psR[64 * lo:64 * lo + 64],
                                scalar1=bqk_sb[0:64, hb:hb + 1])
                            nc.any.tensor_scalar_add(
                                out=dst[64:128, h * T + cl:h * T + cl + QCH],
                                in0=psI[64 * lo:64 * lo + 64],
                                scalar1=bqk_sb[64:128, hb:hb + 1])

            # V token-major: psum [tok(128), CW]; bias added in the
            # psum->SBUF downcast via a prebroadcast fp32 bias tile.
            for s in range(KT):
                sl = s * P
                pvr = ps_v.tile([P, CW], F32, name="pvr", tag="pv")
                for d in range(DT):
                    nc.tensor.matmul(pvr, xr(d, sl, P), w(4, d, 0, CW),
                                     start=(d == 0), stop=(d == DT - 1))
                pvi = ps_v.tile([P, CW], F32, name="pvi", tag="pv")
                for d in range(DT):
                    nc.tensor.matmul(pvi, xi(d, sl, P), w(5, d, 0, CW),
                                     start=(d == 0), stop=(d == DT - 1))
                nc.any.tensor_add(out=vr_sb[:, s * CW:(s + 1) * CW],
                                  in0=pvr, in1=bvr_bc)
                nc.any.tensor_add(out=vi_sb[:, s * CW:(s + 1) * CW],
                                  in0=pvi, in1=bvi_bc)

        # ================= Phase 2: causal attention =================
        # Software-pipelined over the flat (h, j, k) tile list: the S^T
        # matmul of tile i+1 is issued before the pl/po consumers of tile i,
        # hiding the PE -> ACT(exp) -> gpsimd(mask) -> PE latency per tile.
        # The per-(h,j) normalization chain is deferred one tile so the PE's
        # K=1 broadcast matmul never waits on the DVE reciprocal; the
        # reciprocal itself is issued immediately (ps_l has bufs=1, so its
        # read must precede the next group's pl write in program order).
        with ExitStack() as ctx:
            epool = ctx.enter_context(tc.tile_pool(name="epool", bufs=6))
            rpool = ctx.enter_context(tc.tile_pool(name="rpool", bufs=2))
            ps_s = ctx.enter_context(tc.tile_pool(name="ps_s", bufs=2, space="PSUM"))
            ps_o = ctx.enter_context(tc.tile_pool(name="ps_o", bufs=2, space="PSUM"))
            ps_l = ctx.enter_context(tc.tile_pool(name="ps_l", bufs=1, space="PSUM"))
            ps_b = ctx.enter_context(tc.tile_pool(name="ps_b", bufs=1, space="PSUM"))

            tiles = []
            if 2 in phases:
                for h in range(HPC):
                    for j in range(QC):
                        nk = (j + 1) * QKB
                        for k in range(nk):
                            tiles.append((h, j, k, k == 0, k == nk - 1))
            NT = len(tiles)

            ets = {}
            grp = {}
            pending = []

            def stage_a(i):
                h, j, k, first, last = tiles[i]
                st = ps_s.tile([P, QCH], F32, name="st")
                nc.tensor.matmul(
                    st, k_sb[:, h * T + k * P:h * T + (k + 1) * P],
                    q_sb[:, h * T + j * QCH:h * T + (j + 1) * QCH],
                    start=True, stop=True)
                et = epool.tile([P, QCH], BF16, name="et")
                nc.scalar.activation(
                    et, st, mybir.ActivationFunctionType.Exp, scale=scale)
                if k >= j * QKB:
                    # keep where qtok >= ktok: -p + f + (QCH*j - 128*k) >= 0
                    nc.gpsimd.affine_select(
                        out=et, in_=et,
                        compare_op=mybir.AluOpType.is_ge,
                        fill=0.0,
                        base=QCH * j - P * k,
                        pattern=[[1, QCH]],
                        channel_multiplier=-1)
                ets[i] = et

            def stage_b(i):
                h, j, k, first, last = tiles[i]
                pair, lo = h // 2, h % 2
                base_r = 64 * lo          # vr rows in ORT[pair]
                base_i = 64 * (1 - lo)    # vi rows in OIT[pair] (swapped)
                et = ets.pop(i)
                if first:
                    grp["po_r"] = ps_o.tile([P, QCH], F32, name="po_r")
                    grp["po_i"] = ps_o.tile([P, QCH], F32, name="po_i")
                    grp["pl"] = ps_l.tile([1, QCH], F32, name="pl")
                po_r, po_i, pl = grp["po_r"], grp["po_i"], grp["pl"]
                nc.tensor.matmul(pl, ones_col, et, start=first, stop=last)
                nc.tensor.matmul(
                    po_r[base_r:base_r + 64],
                    vr_sb[:, k * CW + h * DH:k * CW + (h + 1) * DH],
                    et, start=first, stop=last, tile_position=(0, base_r))
                nc.tensor.matmul(
                    po_i[base_i:base_i + 64],
                    vi_sb[:, k * CW + h * DH:k * CW + (h + 1) * DH],
                    et, start=first, stop=last, tile_position=(0, base_i))
                if last:
                    rl = rpool.tile([1, QCH], F32, name="rl")
                    nc.vector.reciprocal(rl, pl)
                    pending.append((h, j, po_r, po_i, rl))

            def finalize(h, j, po_r, po_i, rl):
                pair, lo = h // 2, h % 2
                base_r, base_i = 64 * lo, 64 * (1 - lo)
                pb = ps_b.tile([P, QCH], F32, name="pb")
                nc.tensor.matmul(pb, ones_row, rl, start=True, stop=True)
                sb_b = rpool.tile([P, QCH], F32, name="sb_b")
                nc.any.tensor_copy(out=sb_b, in_=pb)
                qs = slice(j * QCH, (j + 1) * QCH)
                nc.any.tensor_mul(
                    out=ort[pair][base_r:base_r + 64, qs],
                    in0=po_r[base_r:base_r + 64],
                    in1=sb_b[base_r:base_r + 64])
                nc.any.tensor_mul(
                    out=oit[pair][base_i:base_i + 64, qs],
                    in0=po_i[base_i:base_i + 64],
                    in1=sb_b[base_i:base_i + 64])

            if NT:
                stage_a(0)
            for i in range(NT):
                if i + 1 < NT:
                    stage_a(i + 1)
                stage_b(i)
                while len(pending) > 1:
                    finalize(*pending.pop(0))
            while pending:
                finalize(*pending.pop(0))

        # ================= Phase 3: output projection =================
        with ExitStack() as ctx:
            sout = ctx.enter_context(tc.tile_pool(name="sout", bufs=3))
            ps_f = ctx.enter_context(tc.tile_pool(name="ps_f", bufs=2, space="PSUM"))

            NC2 = D // QCH
            for (oblocks, widx, odst) in (
                ((ort, 0, out_r_t), (oit, 1, out_i_t)) if 3 in phases else ()
            ):
                for t in range(KT):
                    # one 2-bank psum spanning the full D: one copy + one DMA
                    # with 2KB-per-partition lines (full DMA efficiency)
                    pf = ps_f.tile([P, D], F32, name="pf")
                    for n in range(NC2):
                        for kk in range(NPAIR):
                            nc.tensor.matmul(
                                pf[:, n * QCH:(n + 1) * QCH],
                                oblocks[kk][:, t * P:(t + 1) * P],
                                wo(widx, kk, n * QCH, QCH),
                                start=(kk == 0), stop=(kk == NPAIR - 1))
                    ot = sout.tile([P, D], BF16, name="ot")
                    nc.any.tensor_copy(out=ot, in_=pf)
                    nc.sync.dma_start(odst[:, t, :], ot)

    nc.compile()
    return nc


def _to_sbuf_image(a, rows):
    """[rows*128, m] -> [128, rows, m] partition-major, flattened to
    [128, rows*m] (matches AP.rearrange('(t p) m -> p t m'))."""
    m = a.shape[1]
    return a.reshape(rows, P, m).transpose(1, 0, 2).reshape(P, rows * m)


def make_core_inputs(inputs, cfg=None):
    """Slice full inputs into 8 per-core input maps (bf16 SBUF-image blob +
    fp32 biases)."""
    bf16 = mybir.dt.np(BF16)
    f = lambda a: np.asarray(a, dtype=np.float32)
    xt = {}
    for b in range(B):
        xt[(b, "r")] = _to_sbuf_image(
            np.ascontiguousarray(f(inputs["x_real"])[b].T).astype(bf16), DT)
        xt[(b, "i")] = _to_sbuf_image(
            np.ascontiguousarray(f(inputs["x_imag"])[b].T).astype(bf16), DT)
    maps = []
    for c in range(N_CORES):
        b = c // 4
        g = c % 4
        cs = slice(g * CW, (g + 1) * CW)
        parts = [xt[(b, "r")], xt[(b, "i")]]
        for nm in ("Wqr", "Wqi", "Wkr", "Wki", "Wvr", "Wvi"):
            parts.append(_to_sbuf_image(
                np.ascontiguousarray(f(inputs[nm])[:, cs]).astype(bf16), DT))
        wor = f(inputs["Wor"])[cs, :]
        woi = f(inputs["Woi"])[cs, :]
        # OIT pair rows are [h_odd ; h_even] -> permute wo_i rows to match
        woi_perm = np.concatenate(
            [np.concatenate([woi[2 * p * DH + DH:2 * p * DH + 2 * DH],
                             woi[2 * p * DH:2 * p * DH + DH]])
             for p in range(NPAIR)])
        parts.append(_to_sbuf_image(np.ascontiguousarray(wor).astype(bf16),
                                    NPAIR))
        parts.append(_to_sbuf_image(np.ascontiguousarray(woi_perm).astype(bf16),
                                    NPAIR))
        blob = np.ascontiguousarray(np.concatenate(parts, axis=1))
        assert blob.shape == (P, BLOB_N), blob.shape

        bqr, bqi = f(inputs["bqr"])[cs], f(inputs["bqi"])[cs]
        bkr, bki = f(inputs["bkr"])[cs], f(inputs["bki"])[cs]
        bq_t = np.stack(
            [np.concatenate([bqr[h * DH:(h + 1) * DH], bqi[h * DH:(h + 1) * DH]])
             for h in range(HPC)], axis=1)
        bk_t = np.stack(
            [np.concatenate([bkr[h * DH:(h + 1) * DH], bki[h * DH:(h + 1) * DH]])
             for h in range(HPC)], axis=1)
        bqk = np.ascontiguousarray(
            np.concatenate([bq_t, bk_t], axis=1).astype(np.float32))
        bvv = np.ascontiguousarray(np.concatenate(
            [f(inputs["bvr"])[cs], f(inputs["bvi"])[cs]])[None, :])
        maps.append({"blob": blob, "bqk": bqk, "bv": bvv})
    return maps


def kernel(**inputs):
    global LAST
    nc = build_program()
    in_maps = make_core_inputs(inputs)
    res = run_bass_kernel_spmd(
        nc, in_maps, core_ids=list(range(N_CORES)), trace=TRACE)
    LAST = {"exec_time_ns": res.exec_time_ns,
            "trace": res.instructions_and_trace,
            "profile_json": res.profile_json,
            "nc": nc}
    f = lambda a: np.asarray(a, dtype=np.float32)
    bor, boi = f(inputs["bor"]), f(inputs["boi"])
    final_r = np.stack([
        sum(f(res.results[c]["out_r"]) for c in range(4 * b, 4 * b + 4)) + bor
        for b in range(B)]).astype(np.float32)
    final_i = np.stack([
        sum(f(res.results[c]["out_i"]) for c in range(4 * b, 4 * b + 4)) + boi
        for b in range(B)]).astype(np.float32)
    return final_r, final_i
